# revision 1
# baseline (speedup 1.0000x reference)
"""GCNII-with-JK distributed Trainium2 kernel (8 NeuronCores).

Strategy (hardcoded for N=100000, E=1600000, H=128, L=8):
  - Nodes dst-sharded across 8 cores (12500/core, padded to 12544 = 98 windows x 128).
  - Per-core node->window assignment balanced by degree (LPT).
  - Edge gather: gpsimd dma_gather (batched SWDGE descriptors) from a DRAM
    z-table replicated via AllGather each layer. int16 gather indices =>
    table split into 4 src-range buckets (base-offset slices); edges
    scheduled per (window, bucket) into 128-edge chunks; one gather
    instruction per (4-window group, bucket).
  - Segment-sum scatter: one-hot matmuls accumulated in PSUM.
      layer0 (GCNConv) scheme A: out = onehot^T @ G -> [dst, feat] node-major.
      layers 1..8 scheme B:      out = G^T @ onehot -> [feat, dst] feature-major,
      feeding z = (0.9*agg + 0.1*x0) @ W'_i as two accumulated matmuls with
      host-folded weights W'_i = (1-beta_i) I + beta_i conv_w[i].
  - BN(eval)+relu folded as replicated row constants; JK 'max' every 4 layers.
"""
import sys
sys.path.insert(0, "/opt/trn_rl_repo")
import hashlib
import heapq
import numpy as np

N, E, H, L = 100000, 1600000, 128, 8
ALPHA, THETA, BN_EPS = 0.1, 0.5, 1e-5
C = 8
NS = N // C          # 12500
P = 128
NW = 98              # windows per core
NS_PAD = NW * P      # 12544
NT = C * NS_PAD      # 100352 table rows
HALF = 49 * P            # 6272 rows: first 49 windows of each core
TH = C * HALF            # 50176 rows per table half
TBASE = [0, 32768, 0, 32768]          # base row within its half-tensor
THALF = [0, 0, 1, 1]                  # which half-tensor each bucket reads
BUCK_GLOBAL = [0, 32768, TH, TH + 32768, 2 * TH]
NB = 4
GW = 4               # windows per gather group
NG = (NW + GW - 1) // GW


# ----------------------------------------------------------------- host prep
def _host_prep(edge_index):
    src = np.asarray(edge_index[0]).astype(np.int64)
    dst = np.asarray(edge_index[1]).astype(np.int64)
    deg = np.bincount(dst, minlength=N).astype(np.float32) + 1.0
    dinv = (1.0 / np.sqrt(deg)).astype(np.float32)

    # balanced node->(window,slot) assignment per core (LPT on degree)
    perm_pos = np.empty(N, np.int64)
    for c in range(C):
        nodes = np.arange(c * NS, (c + 1) * NS)
        d = deg[nodes] - 1.0
        order = np.argsort(-d, kind="stable")
        wcnt = np.zeros(NW, np.int64)
        heap = [(0.0, w) for w in range(NW)]
        heapq.heapify(heap)
        pos = np.empty(NS, np.int64)
        for n_i in order:
            while True:
                s, w = heapq.heappop(heap)
                if wcnt[w] < P:
                    break
            pos[n_i] = w * P + wcnt[w]
            wcnt[w] += 1
            heapq.heappush(heap, (s + d[n_i], w))
        perm_pos[nodes] = c * NS_PAD + pos

    # table layout: [cores' first 49 windows; cores' last 49 windows]
    lp_all = perm_pos % NS_PAD
    cr_all = perm_pos // NS_PAD
    table_pos = np.where(lp_all < HALF, cr_all * HALF + lp_all,
                         TH + cr_all * HALF + (lp_all - HALF))
    src_tp = table_pos[src]
    dst_pos = perm_pos[dst]
    e_core = dst_pos // NS_PAD
    e_w = (dst_pos % NS_PAD) // P
    e_slot = dst_pos % P
    e_b = np.searchsorted(np.asarray(BUCK_GLOBAL), src_tp, side="right") - 1
    e_loc = src_tp - np.asarray(BUCK_GLOBAL)[e_b]    # local idx within bucket

    # group edges by (core, window, bucket)
    key = (e_core * NW + e_w) * NB + e_b
    order = np.argsort(key, kind="stable")
    key_s = key[order]
    nkey = C * NW * NB
    grp_start = np.searchsorted(key_s, np.arange(nkey))
    grp_end = np.searchsorted(key_s, np.arange(nkey) + 1)
    counts = (grp_end - grp_start).reshape(C, NW, NB)
    # chunks per (window, bucket): max over cores (SPMD-uniform)
    n_wb = (counts.max(axis=0) + P - 1) // P         # [NW, NB]

    # --- schedule ---------------------------------------------------------
    S_w = n_wb.sum(axis=1)                           # chunks per window
    max_S = int(S_w.max())
    slot_base = np.concatenate([[0], np.cumsum(S_w)]).astype(np.int64)
    SLOT_TOT = int(slot_base[-1])
    # per group/bucket chunk counts and offsets inside the group G tile
    groups = [list(range(g * GW, min((g + 1) * GW, NW))) for g in range(NG)]
    gb_chunks = [[int(n_wb[ws, b].sum()) for b in range(NB)] for ws in
                 [np.array(g) for g in groups]]
    # G-tile column (chunk) index for (w, k): bucket-major inside group
    gcol = {}
    for gi, g in enumerate(groups):
        off = 0
        for b in range(NB):
            for w in g:
                k0 = int(n_wb[w, :b].sum())
                for k in range(int(n_wb[w, b])):
                    gcol[(w, k0 + k)] = off
                    off += 1
    max_gchunks = max(sum(cb) for cb in gb_chunks)

    # --- tables -----------------------------------------------------------
    dstloc = np.full((P, SLOT_TOT), -1.0, np.float32)
    # idx16 columns: per (g, b) range of num_idxs/16 columns
    idx_cols = [[cb * P // 16 for cb in cbs] for cbs in gb_chunks]
    col_base = {}
    acc = 0
    for gi in range(NG):
        for b in range(NB):
            col_base[(gi, b)] = acc
            acc += idx_cols[gi][b]
    TOTCOL = acc
    idx16 = np.zeros((C, P, TOTCOL), np.int16)

    # order edges within each (c, w, b) group; scatter into tables
    k_in_grp = np.arange(E) - grp_start[key_s]
    oc = key_s // (NW * NB)
    ow = (key_s // NB) % NW
    ob = key_s % NB
    # dstloc: slot position within window = (bucket chunk offset + chunk)*?:
    wb_chunk_off = np.concatenate(
        [np.zeros((NW, 1), np.int64), np.cumsum(n_wb, axis=1)[:, :-1]], axis=1)
    slot_in_w = wb_chunk_off[ow, ob] + k_in_grp // P       # chunk within window
    dl_col = slot_base[ow] + slot_in_w
    lane = k_in_grp % P
    # NOTE: dstloc identical construction per core -> index with (lane, col) per core
    for c in range(C):
        pass  # dstloc is per-core; fill below
    dstloc_all = np.full((C, P, SLOT_TOT), -1.0, np.float32)
    dstloc_all[oc, lane, dl_col] = e_slot[order].astype(np.float32)

    # idx16: j ordering per (g,b): chunks bucket-major across windows in group
    gi_of_w = np.array([w // GW for w in range(NW)])
    ogi = gi_of_w[ow]
    # chunk offset of (w,b) within its (g,b) gather region:
    wb_in_gb_off = np.zeros((NW, NB), np.int64)
    for gi, g in enumerate(groups):
        for b in range(NB):
            off = 0
            for w in g:
                wb_in_gb_off[w, b] = off
                off += int(n_wb[w, b])
    j_idx = (wb_in_gb_off[ow, ob] + k_in_grp // P) * P + lane
    jc = j_idx // 16
    jr = j_idx % 16
    cb = np.array([[col_base[(gi, b)] for b in range(NB)] for gi in range(NG)])
    cols = cb[ogi, ob] + jc
    for r in range(8):
        idx16[oc, 16 * r + jr, cols] = e_loc[order].astype(np.int16)

    dinv_pad = np.ones((C, P, NW), np.float32)
    lp = perm_pos % NS_PAD
    dinv_pad[perm_pos // NS_PAD, lp % P, lp // P] = dinv

    sched = dict(
        n_wb=[[int(v) for v in row] for row in n_wb],
        S_w=[int(v) for v in S_w],
        slot_base=[int(v) for v in slot_base],
        max_S=max_S, SLOT_TOT=SLOT_TOT,
        gb_chunks=gb_chunks, col_base={f"{g}_{b}": v for (g, b), v in col_base.items()},
        TOTCOL=TOTCOL, max_gchunks=int(max_gchunks),
        groups=groups,
        gcol={f"{w}_{k}": v for (w, k), v in gcol.items()},
    )
    return dict(perm_pos=perm_pos, idx16=idx16, dstloc=dstloc_all,
                dinv=dinv_pad, sched=sched)


def _host_consts(inputs):
    w0 = np.asarray(inputs["w0"], np.float32)
    b0 = np.asarray(inputs["b0"], np.float32)
    conv_w = np.asarray(inputs["conv_w"], np.float32)
    bn_gamma = np.asarray(inputs["bn_gamma"], np.float32)
    bn_beta = np.asarray(inputs["bn_beta"], np.float32)
    bn_scale = bn_gamma / np.float32(np.sqrt(1.0 + BN_EPS))

    wp09 = np.zeros((P, L * P), np.float32)
    wp01 = np.zeros((P, L * P), np.float32)
    bn_s = np.zeros((P, L * P), np.float32)
    bn_b = np.zeros((P, L * P), np.float32)
    eye = np.eye(H, dtype=np.float32)
    for i in range(L):
        beta = np.float32(np.log(THETA / (i + 1) + 1.0))
        Wp = (1.0 - beta) * eye + beta * conv_w[i]
        wp09[:, i * P:(i + 1) * P] = np.float32(1.0 - ALPHA) * Wp
        wp01[:, i * P:(i + 1) * P] = np.float32(ALPHA) * Wp
        bn_s[:, i * P:(i + 1) * P] = np.broadcast_to(bn_scale[i], (P, H))
        bn_b[:, i * P:(i + 1) * P] = np.broadcast_to(bn_beta[i], (P, H))
    b0r = np.broadcast_to(b0, (P, H)).astype(np.float32).copy()
    iota = np.broadcast_to(np.arange(P, dtype=np.float32), (P, P)).copy()
    return dict(w0=w0, wp09=wp09, wp01=wp01, bn_s=bn_s, bn_b=bn_b, b0r=b0r,
                iota=iota)


# ------------------------------------------------------------ device program
def _build_program(sched):
    from concourse import bass, bacc, mybir, tile
    from concourse.masks import make_identity

    f32 = mybir.dt.float32
    bf16 = mybir.dt.bfloat16
    i16 = mybir.dt.int16
    Alu = mybir.AluOpType
    Act = mybir.ActivationFunctionType

    S_w = sched["S_w"]
    slot_base = sched["slot_base"]
    SLOT_TOT = sched["SLOT_TOT"]
    max_S = sched["max_S"]
    gb_chunks = sched["gb_chunks"]
    col_base = {tuple(map(int, k.split("_"))): v for k, v in sched["col_base"].items()}
    TOTCOL = sched["TOTCOL"]
    max_gchunks = sched["max_gchunks"]
    groups = sched["groups"]
    gcol = {tuple(map(int, k.split("_"))): v for k, v in sched["gcol"].items()}

    nc = bacc.Bacc("TRN2", target_bir_lowering=False, debug=False, num_devices=C,
                   num_swdge_queues=4)

    xs_io = nc.dram_tensor("xs", [NS_PAD, H], f32, kind="ExternalInput")
    idx_io = nc.dram_tensor("idx16", [P, TOTCOL], i16, kind="ExternalInput")
    dstloc_io = nc.dram_tensor("dstloc", [P, SLOT_TOT], bf16, kind="ExternalInput")
    dinv_io = nc.dram_tensor("dinv", [P, NW], f32, kind="ExternalInput")
    w0_io = nc.dram_tensor("w0", [P, H], f32, kind="ExternalInput")
    wp09_io = nc.dram_tensor("wp09", [P, L * P], f32, kind="ExternalInput")
    wp01_io = nc.dram_tensor("wp01", [P, L * P], f32, kind="ExternalInput")
    bn_s_io = nc.dram_tensor("bn_s", [P, L * P], f32, kind="ExternalInput")
    bn_b_io = nc.dram_tensor("bn_b", [P, L * P], f32, kind="ExternalInput")
    b0r_io = nc.dram_tensor("b0r", [P, H], f32, kind="ExternalInput")
    iota_io = nc.dram_tensor("iota", [P, P], bf16, kind="ExternalInput")
    out_io = nc.dram_tensor("out", [NS_PAD, H], f32, kind="ExternalOutput")

    with tile.TileContext(nc) as tc:
        with (
            tc.tile_pool(name="const", bufs=1) as cpool,
            tc.tile_pool(name="big", bufs=1) as bigpool,
            tc.tile_pool(name="gbuf", bufs=3) as gpool,
            tc.tile_pool(name="ohbuf", bufs=3) as ohpool,
            tc.tile_pool(name="ixbuf", bufs=3) as ixpool,
            tc.tile_pool(name="win", bufs=4) as wpool,
            tc.tile_pool(name="ps", bufs=2, space="PSUM") as ps,
            tc.tile_pool(name="dram", bufs=1, space="DRAM") as dram,
        ):
            dstloc_t = cpool.tile([P, SLOT_TOT], bf16, name="dstloc_t")
            dinv_t = cpool.tile([P, NW], f32, name="dinv_t")
            w0_t = cpool.tile([P, H], f32, name="w0_t")
            wp09_t = cpool.tile([P, L * P], f32, name="wp09_t")
            wp01_t = cpool.tile([P, L * P], f32, name="wp01_t")
            bn_s_t = cpool.tile([P, L * P], f32, name="bn_s_t")
            bn_b_t = cpool.tile([P, L * P], f32, name="bn_b_t")
            b0r_t = cpool.tile([P, H], f32, name="b0r_t")
            iota_t = cpool.tile([P, P], bf16, name="iota_t")
            ident_t = cpool.tile([P, P], f32, name="ident_t")
            for t, io in [(dstloc_t, dstloc_io), (dinv_t, dinv_io),
                          (w0_t, w0_io), (wp09_t, wp09_io), (wp01_t, wp01_io),
                          (bn_s_t, bn_s_io), (bn_b_t, bn_b_io),
                          (b0r_t, b0r_io), (iota_t, iota_io)]:
                nc.sync.dma_start(t[:], io[:])
            make_identity(nc, ident_t[:])

            x0T = bigpool.tile([P, NS_PAD], f32, name="x0T")

            tables = [(dram.tile([TH, H], bf16, addr_space="Shared", name=f"tableA{i}"),
                       dram.tile([TH, H], bf16, addr_space="Shared", name=f"tableB{i}"))
                      for i in range(L + 1)]
            agbufs = [(dram.tile([HALF, H], bf16, name=f"agbufA{i}"),
                       dram.tile([NS_PAD - HALF, H], bf16, name=f"agbufB{i}"))
                      for i in range(L + 1)]
            zsbufs = {i: dram.tile([NS_PAD, H], f32, name=f"zsbuf{i}")
                      for i in (0, 1, 2, 4, 5, 6)}
            hd2buf = dram.tile([NS_PAD, H], f32, name="hd2buf")

            RG = [list(range(C))]

            def allgather(i, half):
                nc.gpsimd.collective_compute(
                    "AllGather", Alu.bypass, replica_groups=RG,
                    ins=[agbufs[i][half].opt()], outs=[tables[i][half].opt()])

            def agwrite(i, ws, tile_):
                # route a window's AG-input write to the right half buffer
                w0 = ws.start // P
                if w0 < 49:
                    nc.sync.dma_start(agbufs[i][0][ws], tile_[:])
                else:
                    hs = slice(ws.start - HALF, ws.stop - HALF)
                    nc.sync.dma_start(agbufs[i][1][hs], tile_[:])
                if w0 == 48:
                    allgather(i, 0)
                elif w0 == NW - 1:
                    allgather(i, 1)

            qctr = [0]

            def gather_group(table, gi):
                """One dma_gather per bucket for the 4-window group gi."""
                cbs = gb_chunks[gi]
                gcols = sum(cbs)
                g = gpool.tile([P, max_gchunks * P], bf16, name="g")
                c0 = col_base[(gi, 0)]
                ctot = sum(cbs) * P // 16
                ix = ixpool.tile([P, max_gchunks * P // 16], i16, name="ix")
                nc.sync.dma_start(ix[:, :ctot], idx_io[:, c0:c0 + ctot])
                off = 0
                for b in range(NB):
                    nch = cbs[b]
                    if nch == 0:
                        continue
                    icol0 = col_base[(gi, b)] - c0
                    # SWDGE ring limit: <=1024 indices (64 descs/engine) per op
                    for s0 in range(0, nch, 8):
                        sn = min(8, nch - s0)
                        sl = g[:, (off + s0) * P:(off + s0 + sn) * P]
                        out_ap = bass.AP(sl.tensor, sl.offset,
                                         [list(sl.ap[0]), [P, sn], [1, P]])
                        ic = icol0 + s0 * P // 16
                        tsrc = table[THALF[b]]
                        nc.gpsimd.dma_gather(
                            out_ap=out_ap,
                            in_ap=tsrc[TBASE[b]:TBASE[b] + (BUCK_GLOBAL[b + 1] - BUCK_GLOBAL[b])],
                            idxs_ap=ix[:, ic:ic + sn * P // 16],
                            num_idxs=sn * P, num_idxs_reg=sn * P, elem_size=H,
                            queue_num=qctr[0] % 4)
                        qctr[0] += 1
                    off += nch
                return g

            def onehot_window(w):
                sw = S_w[w]
                oh = ohpool.tile([P, max_S * P], bf16, name="oh")
                src = dstloc_t[:, slot_base[w]:slot_base[w] + sw]
                in0 = src.to_broadcast([P, sw, P])
                io_ap = iota_t[:]
                in1 = bass.AP(io_ap.tensor, io_ap.offset,
                              [list(io_ap.ap[0]), [0, sw], [1, P]])
                nc.vector.tensor_tensor(out=oh[:, :sw * P], in0=in0, in1=in1,
                                        op=Alu.is_equal)
                return oh

            # ================= Phase A: h' = (x @ w0) * dinv =================
            for w in range(NW):
                ws = slice(w * P, (w + 1) * P)
                xw = wpool.tile([P, H], f32, name="xw")
                nc.sync.dma_start(xw[:], xs_io[ws])
                xT_ps = ps.tile([P, P], f32, name="xT_ps", tag="tr")
                nc.tensor.transpose(out=xT_ps[:], in_=xw[:], identity=ident_t[:])
                xT = wpool.tile([P, P], f32, name="xT")
                nc.vector.tensor_copy(out=xT[:], in_=xT_ps[:])
                h_ps = ps.tile([P, H], f32, name="h_ps", tag="mm")
                nc.tensor.matmul(out=h_ps[:], lhsT=xT[:], rhs=w0_t[:],
                                 start=True, stop=True)
                dcol = dinv_t[:, w:w + 1]
                hp = wpool.tile([P, H], f32, name="hp")
                nc.vector.tensor_scalar_mul(hp[:], h_ps[:], dcol)
                hd2b = wpool.tile([P, H], f32, name="hd2b")
                nc.vector.scalar_tensor_tensor(
                    out=hd2b[:], in0=hp[:], scalar=dcol, in1=b0r_t[:],
                    op0=Alu.mult, op1=Alu.add)
                hpb = wpool.tile([P, H], bf16, name="hpb")
                nc.vector.tensor_copy(out=hpb[:], in_=hp[:])
                agwrite(0, ws, hpb)
                nc.scalar.dma_start(hd2buf[ws], hd2b[:])

            # ============ Phase B: z0 = dinv*segsum(h'[src]) + h*dinv^2 + b0
            for gi, grp in enumerate(groups):
                g = gather_group(tables[0], gi)
                for w in grp:
                    ws = slice(w * P, (w + 1) * P)
                    oh = onehot_window(w)
                    s_ps = ps.tile([P, H], f32, name="s_ps", tag="acc", bufs=3)
                    for k in range(S_w[w]):
                        cg = gcol[(w, k)]
                        nc.tensor.matmul(out=s_ps[:],
                                         lhsT=oh[:, k * P:(k + 1) * P],
                                         rhs=g[:, cg * P:(cg + 1) * P],
                                         start=(k == 0), stop=(k == S_w[w] - 1))
                    hd2w = wpool.tile([P, H], f32, name="hd2w")
                    nc.scalar.dma_start(hd2w[:], hd2buf[ws])
                    z0 = wpool.tile([P, H], f32, name="z0")
                    nc.vector.scalar_tensor_tensor(
                        out=z0[:], in0=s_ps[:], scalar=dinv_t[:, w:w + 1],
                        in1=hd2w[:], op0=Alu.mult, op1=Alu.add)
                    z0b = wpool.tile([P, H], bf16, name="z0b")
                    nc.vector.tensor_copy(out=z0b[:], in_=z0[:])
                    agwrite(1, ws, z0b)
                    zT_ps = ps.tile([P, P], f32, name="zT_ps", tag="tr")
                    nc.tensor.transpose(out=zT_ps[:], in_=z0[:], identity=ident_t[:])
                    nc.vector.tensor_copy(out=x0T[:, ws], in_=zT_ps[:])

            # =========================== Phase C: 8 GCN2 layers
            for i in range(L):
                lsl = slice(i * P, (i + 1) * P)
                for gi, grp in enumerate(groups):
                    g = gather_group(tables[i + 1], gi)
                    for w in grp:
                        ws = slice(w * P, (w + 1) * P)
                        oh = onehot_window(w)
                        st_ps = ps.tile([P, P], f32, name="st_ps", tag="acc", bufs=3)
                        for k in range(S_w[w]):
                            cg = gcol[(w, k)]
                            nc.tensor.matmul(out=st_ps[:],
                                             lhsT=g[:, cg * P:(cg + 1) * P],
                                             rhs=oh[:, k * P:(k + 1) * P],
                                             start=(k == 0), stop=(k == S_w[w] - 1))
                        st = wpool.tile([P, P], f32, name="st")
                        nc.vector.tensor_copy(out=st[:], in_=st_ps[:])
                        z_ps = ps.tile([P, H], f32, name="z_ps", tag="mm")
                        nc.tensor.matmul(out=z_ps[:], lhsT=st[:],
                                         rhs=wp09_t[:, lsl], start=True, stop=False)
                        nc.tensor.matmul(out=z_ps[:], lhsT=x0T[:, ws],
                                         rhs=wp01_t[:, lsl], start=False, stop=True)
                        if i in (3, 7):
                            m = wpool.tile([P, H], f32, name="m")
                            nc.vector.tensor_copy(out=m[:], in_=z_ps[:])
                            for j in range(4 * (i // 4), 4 * (i // 4) + 3):
                                zl = wpool.tile([P, H], f32, name="zl")
                                nc.scalar.dma_start(zl[:], zsbufs[j][ws])
                                nc.vector.tensor_max(m[:], m[:], zl[:])
                            if i == 3:
                                mb_ = wpool.tile([P, H], bf16, name="mb_")
                                nc.vector.tensor_copy(out=mb_[:], in_=m[:])
                                agwrite(i + 2, ws, mb_)
                                mT_ps = ps.tile([P, P], f32, name="mT_ps", tag="tr")
                                nc.tensor.transpose(out=mT_ps[:], in_=m[:],
                                                    identity=ident_t[:])
                                nc.vector.tensor_copy(out=x0T[:, ws], in_=mT_ps[:])
                            else:
                                nc.sync.dma_start(out_io[ws], m[:])
                        else:
                            zsb = wpool.tile([P, H], f32, name="zsb")
                            nc.vector.tensor_copy(out=zsb[:], in_=z_ps[:])
                            nc.scalar.dma_start(zsbufs[i][ws], zsb[:])
                            t1 = wpool.tile([P, H], f32, name="t1")
                            nc.vector.tensor_tensor(out=t1[:], in0=z_ps[:],
                                                    in1=bn_s_t[:, lsl], op=Alu.mult)
                            t2 = wpool.tile([P, H], f32, name="t2")
                            nc.vector.tensor_tensor(out=t2[:], in0=t1[:],
                                                    in1=bn_b_t[:, lsl], op=Alu.add)
                            za = wpool.tile([P, H], bf16, name="za")
                            nc.scalar.activation(out=za[:], in_=t2[:], func=Act.Relu)
                            if i < 7:
                                agwrite(i + 2, ws, za)
    nc.finalize()
    return nc


_PROGRAM_CACHE = {}
_PREP_CACHE = {}


def _make_inmaps(prep, consts, inputs):
    import ml_dtypes
    bf = ml_dtypes.bfloat16
    x = np.asarray(inputs["x"], np.float32)
    xp = np.zeros((C * NS_PAD, H), np.float32)
    xp[prep["perm_pos"]] = x
    in_maps = []
    for c in range(C):
        in_maps.append({
            "xs": xp[c * NS_PAD:(c + 1) * NS_PAD],
            "idx16": prep["idx16"][c],
            "dstloc": prep["dstloc"][c].astype(bf),
            "dinv": prep["dinv"][c],
            "w0": consts["w0"], "wp09": consts["wp09"], "wp01": consts["wp01"],
            "bn_s": consts["bn_s"], "bn_b": consts["bn_b"], "b0r": consts["b0r"],
            "iota": consts["iota"].astype(bf),
        })
    return in_maps


def kernel(**inputs) -> np.ndarray:
    from concourse.bass_utils import run_bass_kernel_spmd

    edge_index = np.asarray(inputs["edge_index"])
    ekey = hashlib.md5(edge_index.tobytes()).hexdigest()
    if ekey not in _PREP_CACHE:
        _PREP_CACHE[ekey] = _host_prep(edge_index)
    prep = _PREP_CACHE[ekey]
    skey = hashlib.md5(repr(prep["sched"]).encode()).hexdigest()
    if skey not in _PROGRAM_CACHE:
        _PROGRAM_CACHE[skey] = _build_program(prep["sched"])
    nc = _PROGRAM_CACHE[skey]

    consts = _host_consts(inputs)
    in_maps = _make_inmaps(prep, consts, inputs)
    res = run_bass_kernel_spmd(nc, in_maps, list(range(C)))
    out_cat = np.concatenate([res.results[c]["out"] for c in range(C)], axis=0)
    return out_cat[prep["perm_pos"]]



# revision 3
# speedup vs baseline: 8.8786x; 8.8786x over previous
"""GCNII-with-JK distributed Trainium2 kernel (8 NeuronCores).

Strategy (hardcoded for N=100000, E=1600000, H=128, L=8):
  - Nodes dst-sharded across 8 cores (12500/core, padded to 12544 = 98 windows x 128).
  - Per-core node->window assignment balanced by degree (LPT).
  - Edge gather: gpsimd dma_gather (batched SWDGE descriptors) from a DRAM
    z-table replicated via AllGather each layer. int16 gather indices =>
    table split into 4 src-range buckets (base-offset slices); edges
    scheduled per (window, bucket) into 128-edge chunks; one gather
    instruction per (4-window group, bucket).
  - Segment-sum scatter: one-hot matmuls accumulated in PSUM.
      layer0 (GCNConv) scheme A: out = onehot^T @ G -> [dst, feat] node-major.
      layers 1..8 scheme B:      out = G^T @ onehot -> [feat, dst] feature-major,
      feeding z = (0.9*agg + 0.1*x0) @ W'_i as two accumulated matmuls with
      host-folded weights W'_i = (1-beta_i) I + beta_i conv_w[i].
  - BN(eval)+relu folded as replicated row constants; JK 'max' every 4 layers.
"""
import sys
sys.path.insert(0, "/opt/trn_rl_repo")
import hashlib
import heapq
import numpy as np

N, E, H, L = 100000, 1600000, 128, 8
ALPHA, THETA, BN_EPS = 0.1, 0.5, 1e-5
C = 8
NS = N // C          # 12500
P = 128
NW = 98              # windows per core
NS_PAD = NW * P      # 12544
NT = C * NS_PAD      # 100352 table rows
HALF = 49 * P            # 6272 rows: first 49 windows of each core
TH = C * HALF            # 50176 rows per table half
TBASE = [0, 32768, 0, 32768]          # base row within its half-tensor
THALF = [0, 0, 1, 1]                  # which half-tensor each bucket reads
BUCK_GLOBAL = [0, 32768, TH, TH + 32768, 2 * TH]
NB = 4
GW = 4               # windows per gather group
NG = (NW + GW - 1) // GW
NQ = 4               # SWDGE queues (sim analysis overrides to 1)


# ----------------------------------------------------------------- host prep
def _host_prep(edge_index):
    src = np.asarray(edge_index[0]).astype(np.int64)
    dst = np.asarray(edge_index[1]).astype(np.int64)
    deg = np.bincount(dst, minlength=N).astype(np.float32) + 1.0
    dinv = (1.0 / np.sqrt(deg)).astype(np.float32)

    # balanced node->(window,slot) assignment per core (LPT on degree)
    perm_pos = np.empty(N, np.int64)
    for c in range(C):
        nodes = np.arange(c * NS, (c + 1) * NS)
        d = deg[nodes] - 1.0
        order = np.argsort(-d, kind="stable")
        wcnt = np.zeros(NW, np.int64)
        heap = [(0.0, w) for w in range(NW)]
        heapq.heapify(heap)
        pos = np.empty(NS, np.int64)
        for n_i in order:
            while True:
                s, w = heapq.heappop(heap)
                if wcnt[w] < P:
                    break
            pos[n_i] = w * P + wcnt[w]
            wcnt[w] += 1
            heapq.heappush(heap, (s + d[n_i], w))
        perm_pos[nodes] = c * NS_PAD + pos

    # table layout: [cores' first 49 windows; cores' last 49 windows]
    lp_all = perm_pos % NS_PAD
    cr_all = perm_pos // NS_PAD
    table_pos = np.where(lp_all < HALF, cr_all * HALF + lp_all,
                         TH + cr_all * HALF + (lp_all - HALF))
    src_tp = table_pos[src]
    dst_pos = perm_pos[dst]
    e_core = dst_pos // NS_PAD
    e_w = (dst_pos % NS_PAD) // P
    e_slot = dst_pos % P
    e_b = np.searchsorted(np.asarray(BUCK_GLOBAL), src_tp, side="right") - 1
    e_loc = src_tp - np.asarray(BUCK_GLOBAL)[e_b]    # local idx within bucket

    # group edges by (core, window, bucket)
    key = (e_core * NW + e_w) * NB + e_b
    order = np.argsort(key, kind="stable")
    key_s = key[order]
    nkey = C * NW * NB
    grp_start = np.searchsorted(key_s, np.arange(nkey))
    grp_end = np.searchsorted(key_s, np.arange(nkey) + 1)
    counts = (grp_end - grp_start).reshape(C, NW, NB)
    # chunks per (window, bucket): max over cores (SPMD-uniform)
    n_wb = (counts.max(axis=0) + P - 1) // P         # [NW, NB]

    # --- schedule ---------------------------------------------------------
    S_w = n_wb.sum(axis=1)                           # chunks per window
    max_S = int(S_w.max())
    slot_base = np.concatenate([[0], np.cumsum(S_w)]).astype(np.int64)
    SLOT_TOT = int(slot_base[-1])
    # per group/bucket chunk counts and offsets inside the group G tile
    groups = [list(range(g * GW, min((g + 1) * GW, NW))) for g in range(NG)]
    gb_chunks = [[int(n_wb[ws, b].sum()) for b in range(NB)] for ws in
                 [np.array(g) for g in groups]]
    # G-tile column (chunk) index for (w, k): bucket-major inside group
    gcol = {}
    for gi, g in enumerate(groups):
        off = 0
        for b in range(NB):
            for w in g:
                k0 = int(n_wb[w, :b].sum())
                for k in range(int(n_wb[w, b])):
                    gcol[(w, k0 + k)] = off
                    off += 1
    max_gchunks = max(sum(cb) for cb in gb_chunks)

    # --- tables -----------------------------------------------------------
    dstloc = np.full((P, SLOT_TOT), -1.0, np.float32)
    # idx16 columns: per (g, b) range of num_idxs/16 columns
    idx_cols = [[cb * P // 16 for cb in cbs] for cbs in gb_chunks]
    col_base = {}
    acc = 0
    for gi in range(NG):
        for b in range(NB):
            col_base[(gi, b)] = acc
            acc += idx_cols[gi][b]
    TOTCOL = acc
    idx16 = np.zeros((C, P, TOTCOL), np.int16)

    # order edges within each (c, w, b) group; scatter into tables
    k_in_grp = np.arange(E) - grp_start[key_s]
    oc = key_s // (NW * NB)
    ow = (key_s // NB) % NW
    ob = key_s % NB
    # dstloc: slot position within window = (bucket chunk offset + chunk)*?:
    wb_chunk_off = np.concatenate(
        [np.zeros((NW, 1), np.int64), np.cumsum(n_wb, axis=1)[:, :-1]], axis=1)
    slot_in_w = wb_chunk_off[ow, ob] + k_in_grp // P       # chunk within window
    dl_col = slot_base[ow] + slot_in_w
    lane = k_in_grp % P
    # NOTE: dstloc identical construction per core -> index with (lane, col) per core
    for c in range(C):
        pass  # dstloc is per-core; fill below
    dstloc_all = np.full((C, P, SLOT_TOT), -1.0, np.float32)
    dstloc_all[oc, lane, dl_col] = e_slot[order].astype(np.float32)

    # idx16: j ordering per (g,b): chunks bucket-major across windows in group
    gi_of_w = np.array([w // GW for w in range(NW)])
    ogi = gi_of_w[ow]
    # chunk offset of (w,b) within its (g,b) gather region:
    wb_in_gb_off = np.zeros((NW, NB), np.int64)
    for gi, g in enumerate(groups):
        for b in range(NB):
            off = 0
            for w in g:
                wb_in_gb_off[w, b] = off
                off += int(n_wb[w, b])
    j_idx = (wb_in_gb_off[ow, ob] + k_in_grp // P) * P + lane
    jc = j_idx // 16
    jr = j_idx % 16
    cb = np.array([[col_base[(gi, b)] for b in range(NB)] for gi in range(NG)])
    cols = cb[ogi, ob] + jc
    for r in range(8):
        idx16[oc, 16 * r + jr, cols] = e_loc[order].astype(np.int16)

    dinv_pad = np.ones((C, P, NW), np.float32)
    lp = perm_pos % NS_PAD
    dinv_pad[perm_pos // NS_PAD, lp % P, lp // P] = dinv

    sched = dict(
        n_wb=[[int(v) for v in row] for row in n_wb],
        S_w=[int(v) for v in S_w],
        slot_base=[int(v) for v in slot_base],
        max_S=max_S, SLOT_TOT=SLOT_TOT,
        gb_chunks=gb_chunks, col_base={f"{g}_{b}": v for (g, b), v in col_base.items()},
        TOTCOL=TOTCOL, max_gchunks=int(max_gchunks),
        groups=groups,
        gcol={f"{w}_{k}": v for (w, k), v in gcol.items()},
    )
    return dict(perm_pos=perm_pos, idx16=idx16, dstloc=dstloc_all,
                dinv=dinv_pad, sched=sched)


def _host_consts(inputs):
    w0 = np.asarray(inputs["w0"], np.float32)
    b0 = np.asarray(inputs["b0"], np.float32)
    conv_w = np.asarray(inputs["conv_w"], np.float32)
    bn_gamma = np.asarray(inputs["bn_gamma"], np.float32)
    bn_beta = np.asarray(inputs["bn_beta"], np.float32)
    bn_scale = bn_gamma / np.float32(np.sqrt(1.0 + BN_EPS))

    wp09 = np.zeros((P, L * P), np.float32)
    wp01 = np.zeros((P, L * P), np.float32)
    bn_s = np.zeros((P, L * P), np.float32)
    bn_b = np.zeros((P, L * P), np.float32)
    eye = np.eye(H, dtype=np.float32)
    for i in range(L):
        beta = np.float32(np.log(THETA / (i + 1) + 1.0))
        Wp = (1.0 - beta) * eye + beta * conv_w[i]
        wp09[:, i * P:(i + 1) * P] = np.float32(1.0 - ALPHA) * Wp
        wp01[:, i * P:(i + 1) * P] = np.float32(ALPHA) * Wp
        bn_s[:, i * P:(i + 1) * P] = np.broadcast_to(bn_scale[i], (P, H))
        bn_b[:, i * P:(i + 1) * P] = np.broadcast_to(bn_beta[i], (P, H))
    b0r = np.broadcast_to(b0, (P, H)).astype(np.float32).copy()
    iota = np.broadcast_to(np.arange(P, dtype=np.float32), (P, P)).copy()
    return dict(w0=w0, wp09=wp09, wp01=wp01, bn_s=bn_s, bn_b=bn_b, b0r=b0r,
                iota=iota)


# ------------------------------------------------------------ device program
def _build_program(sched):
    from concourse import bass, bacc, mybir, tile
    from concourse.masks import make_identity

    f32 = mybir.dt.float32
    bf16 = mybir.dt.bfloat16
    i16 = mybir.dt.int16
    Alu = mybir.AluOpType
    Act = mybir.ActivationFunctionType

    S_w = sched["S_w"]
    slot_base = sched["slot_base"]
    SLOT_TOT = sched["SLOT_TOT"]
    max_S = sched["max_S"]
    gb_chunks = sched["gb_chunks"]
    col_base = {tuple(map(int, k.split("_"))): v for k, v in sched["col_base"].items()}
    TOTCOL = sched["TOTCOL"]
    max_gchunks = sched["max_gchunks"]
    groups = sched["groups"]
    gcol = {tuple(map(int, k.split("_"))): v for k, v in sched["gcol"].items()}

    nc = bacc.Bacc("TRN2", target_bir_lowering=False, debug=False, num_devices=C,
                   num_swdge_queues=NQ)

    xs_io = nc.dram_tensor("xs", [NS_PAD, H], f32, kind="ExternalInput")
    idx_io = nc.dram_tensor("idx16", [P, TOTCOL], i16, kind="ExternalInput")
    dstloc_io = nc.dram_tensor("dstloc", [P, SLOT_TOT], bf16, kind="ExternalInput")
    dinv_io = nc.dram_tensor("dinv", [P, NW], f32, kind="ExternalInput")
    w0_io = nc.dram_tensor("w0", [P, H], f32, kind="ExternalInput")
    wp09_io = nc.dram_tensor("wp09", [P, L * P], f32, kind="ExternalInput")
    wp01_io = nc.dram_tensor("wp01", [P, L * P], f32, kind="ExternalInput")
    bn_s_io = nc.dram_tensor("bn_s", [P, L * P], f32, kind="ExternalInput")
    bn_b_io = nc.dram_tensor("bn_b", [P, L * P], f32, kind="ExternalInput")
    b0r_io = nc.dram_tensor("b0r", [P, H], f32, kind="ExternalInput")
    iota_io = nc.dram_tensor("iota", [P, P], bf16, kind="ExternalInput")
    out_io = nc.dram_tensor("out", [NS_PAD, H], f32, kind="ExternalOutput")

    with tile.TileContext(nc) as tc:
        with (
            tc.tile_pool(name="const", bufs=1) as cpool,
            tc.tile_pool(name="big", bufs=1) as bigpool,
            tc.tile_pool(name="gbuf", bufs=3) as gpool,
            tc.tile_pool(name="ohbuf", bufs=3) as ohpool,
            tc.tile_pool(name="ixbuf", bufs=3) as ixpool,
            tc.tile_pool(name="win", bufs=4) as wpool,
            tc.tile_pool(name="ps", bufs=2, space="PSUM") as ps,
            tc.tile_pool(name="dram", bufs=1, space="DRAM") as dram,
        ):
            dstloc_t = cpool.tile([P, SLOT_TOT], bf16, name="dstloc_t")
            dinv_t = cpool.tile([P, NW], f32, name="dinv_t")
            w0_t = cpool.tile([P, H], f32, name="w0_t")
            wp09_t = cpool.tile([P, L * P], f32, name="wp09_t")
            wp01_t = cpool.tile([P, L * P], f32, name="wp01_t")
            bn_s_t = cpool.tile([P, L * P], f32, name="bn_s_t")
            bn_b_t = cpool.tile([P, L * P], f32, name="bn_b_t")
            b0r_t = cpool.tile([P, H], f32, name="b0r_t")
            iota_t = cpool.tile([P, P], bf16, name="iota_t")
            ident_t = cpool.tile([P, P], f32, name="ident_t")
            for t, io in [(dstloc_t, dstloc_io), (dinv_t, dinv_io),
                          (w0_t, w0_io), (wp09_t, wp09_io), (wp01_t, wp01_io),
                          (bn_s_t, bn_s_io), (bn_b_t, bn_b_io),
                          (b0r_t, b0r_io), (iota_t, iota_io)]:
                nc.sync.dma_start(t[:], io[:])
            make_identity(nc, ident_t[:])

            x0T = bigpool.tile([P, NS_PAD], f32, name="x0T")

            tables = [(dram.tile([TH, H], bf16, addr_space="Shared", name=f"tableA{i}"),
                       dram.tile([TH, H], bf16, addr_space="Shared", name=f"tableB{i}"))
                      for i in range(L + 1)]
            agbufs = [(dram.tile([HALF, H], bf16, name=f"agbufA{i}"),
                       dram.tile([NS_PAD - HALF, H], bf16, name=f"agbufB{i}"))
                      for i in range(L + 1)]
            zsbufs = {i: dram.tile([NS_PAD, H], f32, name=f"zsbuf{i}")
                      for i in (0, 1, 2, 4, 5, 6)}
            hd2buf = dram.tile([NS_PAD, H], f32, name="hd2buf")

            RG = [list(range(C))]

            def allgather(i, half):
                nc.gpsimd.collective_compute(
                    "AllGather", Alu.bypass, replica_groups=RG,
                    ins=[agbufs[i][half].opt()], outs=[tables[i][half].opt()])

            def agwrite(i, ws, tile_):
                # route a window's AG-input write to the right half buffer
                w0 = ws.start // P
                if w0 < 49:
                    nc.sync.dma_start(agbufs[i][0][ws], tile_[:])
                else:
                    hs = slice(ws.start - HALF, ws.stop - HALF)
                    nc.sync.dma_start(agbufs[i][1][hs], tile_[:])
                if w0 == 48:
                    allgather(i, 0)
                elif w0 == NW - 1:
                    allgather(i, 1)

            qctr = [0]

            def gather_group(table, gi):
                """One dma_gather per bucket for the 4-window group gi."""
                cbs = gb_chunks[gi]
                gcols = sum(cbs)
                g = gpool.tile([P, max_gchunks * P], bf16, name="g")
                c0 = col_base[(gi, 0)]
                ctot = sum(cbs) * P // 16
                ix = ixpool.tile([P, max_gchunks * P // 16], i16, name="ix")
                nc.sync.dma_start(ix[:, :ctot], idx_io[:, c0:c0 + ctot])
                off = 0
                for b in range(NB):
                    nch = cbs[b]
                    if nch == 0:
                        continue
                    icol0 = col_base[(gi, b)] - c0
                    # SWDGE ring limit: <=1024 indices (64 descs/engine) per op
                    for s0 in range(0, nch, 8):
                        sn = min(8, nch - s0)
                        sl = g[:, (off + s0) * P:(off + s0 + sn) * P]
                        out_ap = bass.AP(sl.tensor, sl.offset,
                                         [list(sl.ap[0]), [P, sn], [1, P]])
                        ic = icol0 + s0 * P // 16
                        tsrc = table[THALF[b]]
                        nc.gpsimd.dma_gather(
                            out_ap=out_ap,
                            in_ap=tsrc[TBASE[b]:TBASE[b] + (BUCK_GLOBAL[b + 1] - BUCK_GLOBAL[b])],
                            idxs_ap=ix[:, ic:ic + sn * P // 16],
                            num_idxs=sn * P, num_idxs_reg=sn * P, elem_size=H,
                            queue_num=qctr[0] % NQ)
                        qctr[0] += 1
                    off += nch
                return g

            def onehot_window(w):
                sw = S_w[w]
                oh = ohpool.tile([P, max_S * P], bf16, name="oh")
                src = dstloc_t[:, slot_base[w]:slot_base[w] + sw]
                in0 = src.to_broadcast([P, sw, P])
                io_ap = iota_t[:]
                in1 = bass.AP(io_ap.tensor, io_ap.offset,
                              [list(io_ap.ap[0]), [0, sw], [1, P]])
                nc.vector.tensor_tensor(out=oh[:, :sw * P], in0=in0, in1=in1,
                                        op=Alu.is_equal)
                return oh

            # ================= Phase A: h' = (x @ w0) * dinv =================
            for w in range(NW):
                ws = slice(w * P, (w + 1) * P)
                xw = wpool.tile([P, H], f32, name="xw")
                nc.sync.dma_start(xw[:], xs_io[ws])
                xT_ps = ps.tile([P, P], f32, name="xT_ps", tag="tr")
                nc.tensor.transpose(out=xT_ps[:], in_=xw[:], identity=ident_t[:])
                xT = wpool.tile([P, P], f32, name="xT")
                nc.vector.tensor_copy(out=xT[:], in_=xT_ps[:])
                h_ps = ps.tile([P, H], f32, name="h_ps", tag="mm")
                nc.tensor.matmul(out=h_ps[:], lhsT=xT[:], rhs=w0_t[:],
                                 start=True, stop=True)
                dcol = dinv_t[:, w:w + 1]
                hp = wpool.tile([P, H], f32, name="hp")
                nc.vector.tensor_scalar_mul(hp[:], h_ps[:], dcol)
                hd2b = wpool.tile([P, H], f32, name="hd2b")
                nc.vector.scalar_tensor_tensor(
                    out=hd2b[:], in0=hp[:], scalar=dcol, in1=b0r_t[:],
                    op0=Alu.mult, op1=Alu.add)
                hpb = wpool.tile([P, H], bf16, name="hpb")
                nc.vector.tensor_copy(out=hpb[:], in_=hp[:])
                agwrite(0, ws, hpb)
                nc.scalar.dma_start(hd2buf[ws], hd2b[:])

            # ============ Phase B: z0 = dinv*segsum(h'[src]) + h*dinv^2 + b0
            for gi, grp in enumerate(groups):
                g = gather_group(tables[0], gi)
                for w in grp:
                    ws = slice(w * P, (w + 1) * P)
                    oh = onehot_window(w)
                    s_ps = ps.tile([P, H], f32, name="s_ps", tag="acc", bufs=3)
                    for k in range(S_w[w]):
                        cg = gcol[(w, k)]
                        nc.tensor.matmul(out=s_ps[:],
                                         lhsT=oh[:, k * P:(k + 1) * P],
                                         rhs=g[:, cg * P:(cg + 1) * P],
                                         start=(k == 0), stop=(k == S_w[w] - 1))
                    hd2w = wpool.tile([P, H], f32, name="hd2w")
                    nc.scalar.dma_start(hd2w[:], hd2buf[ws])
                    z0 = wpool.tile([P, H], f32, name="z0")
                    nc.vector.scalar_tensor_tensor(
                        out=z0[:], in0=s_ps[:], scalar=dinv_t[:, w:w + 1],
                        in1=hd2w[:], op0=Alu.mult, op1=Alu.add)
                    z0b = wpool.tile([P, H], bf16, name="z0b")
                    nc.vector.tensor_copy(out=z0b[:], in_=z0[:])
                    agwrite(1, ws, z0b)
                    zT_ps = ps.tile([P, P], f32, name="zT_ps", tag="tr")
                    nc.tensor.transpose(out=zT_ps[:], in_=z0[:], identity=ident_t[:])
                    nc.vector.tensor_copy(out=x0T[:, ws], in_=zT_ps[:])

            # =========================== Phase C: 8 GCN2 layers
            for i in range(L):
                lsl = slice(i * P, (i + 1) * P)
                for gi, grp in enumerate(groups):
                    g = gather_group(tables[i + 1], gi)
                    for w in grp:
                        ws = slice(w * P, (w + 1) * P)
                        oh = onehot_window(w)
                        st_ps = ps.tile([P, P], f32, name="st_ps", tag="acc", bufs=3)
                        for k in range(S_w[w]):
                            cg = gcol[(w, k)]
                            nc.tensor.matmul(out=st_ps[:],
                                             lhsT=g[:, cg * P:(cg + 1) * P],
                                             rhs=oh[:, k * P:(k + 1) * P],
                                             start=(k == 0), stop=(k == S_w[w] - 1))
                        st = wpool.tile([P, P], f32, name="st")
                        nc.vector.tensor_copy(out=st[:], in_=st_ps[:])
                        z_ps = ps.tile([P, H], f32, name="z_ps", tag="mm")
                        nc.tensor.matmul(out=z_ps[:], lhsT=st[:],
                                         rhs=wp09_t[:, lsl], start=True, stop=False)
                        nc.tensor.matmul(out=z_ps[:], lhsT=x0T[:, ws],
                                         rhs=wp01_t[:, lsl], start=False, stop=True)
                        if i in (3, 7):
                            m = wpool.tile([P, H], f32, name="m")
                            nc.vector.tensor_copy(out=m[:], in_=z_ps[:])
                            for j in range(4 * (i // 4), 4 * (i // 4) + 3):
                                zl = wpool.tile([P, H], f32, name="zl")
                                nc.scalar.dma_start(zl[:], zsbufs[j][ws])
                                nc.vector.tensor_max(m[:], m[:], zl[:])
                            if i == 3:
                                mb_ = wpool.tile([P, H], bf16, name="mb_")
                                nc.vector.tensor_copy(out=mb_[:], in_=m[:])
                                agwrite(i + 2, ws, mb_)
                                mT_ps = ps.tile([P, P], f32, name="mT_ps", tag="tr")
                                nc.tensor.transpose(out=mT_ps[:], in_=m[:],
                                                    identity=ident_t[:])
                                nc.vector.tensor_copy(out=x0T[:, ws], in_=mT_ps[:])
                            else:
                                nc.sync.dma_start(out_io[ws], m[:])
                        else:
                            zsb = wpool.tile([P, H], f32, name="zsb")
                            nc.vector.tensor_copy(out=zsb[:], in_=z_ps[:])
                            nc.scalar.dma_start(zsbufs[i][ws], zsb[:])
                            t1 = wpool.tile([P, H], f32, name="t1")
                            nc.vector.tensor_tensor(out=t1[:], in0=z_ps[:],
                                                    in1=bn_s_t[:, lsl], op=Alu.mult)
                            t2 = wpool.tile([P, H], f32, name="t2")
                            nc.vector.tensor_tensor(out=t2[:], in0=t1[:],
                                                    in1=bn_b_t[:, lsl], op=Alu.add)
                            za = wpool.tile([P, H], bf16, name="za")
                            nc.scalar.activation(out=za[:], in_=t2[:], func=Act.Relu)
                            if i < 7:
                                agwrite(i + 2, ws, za)
    nc.finalize()
    return nc


_PROGRAM_CACHE = {}
_PREP_CACHE = {}


def _make_inmaps(prep, consts, inputs):
    import ml_dtypes
    bf = ml_dtypes.bfloat16
    x = np.asarray(inputs["x"], np.float32)
    xp = np.zeros((C * NS_PAD, H), np.float32)
    xp[prep["perm_pos"]] = x
    in_maps = []
    for c in range(C):
        in_maps.append({
            "xs": xp[c * NS_PAD:(c + 1) * NS_PAD],
            "idx16": prep["idx16"][c],
            "dstloc": prep["dstloc"][c].astype(bf),
            "dinv": prep["dinv"][c],
            "w0": consts["w0"], "wp09": consts["wp09"], "wp01": consts["wp01"],
            "bn_s": consts["bn_s"], "bn_b": consts["bn_b"], "b0r": consts["b0r"],
            "iota": consts["iota"].astype(bf),
        })
    return in_maps


def kernel(**inputs) -> np.ndarray:
    from concourse.bass_utils import run_bass_kernel_spmd

    edge_index = np.asarray(inputs["edge_index"])
    ekey = hashlib.md5(edge_index.tobytes()).hexdigest()
    if ekey not in _PREP_CACHE:
        _PREP_CACHE[ekey] = _host_prep(edge_index)
    prep = _PREP_CACHE[ekey]
    skey = hashlib.md5(repr(prep["sched"]).encode()).hexdigest()
    if skey not in _PROGRAM_CACHE:
        _PROGRAM_CACHE[skey] = _build_program(prep["sched"])
    nc = _PROGRAM_CACHE[skey]

    consts = _host_consts(inputs)
    in_maps = _make_inmaps(prep, consts, inputs)
    res = run_bass_kernel_spmd(nc, in_maps, list(range(C)))
    out_cat = np.concatenate([res.results[c]["out"] for c in range(C)], axis=0)
    return out_cat[prep["perm_pos"]]

